# revision 1
# baseline (speedup 1.0000x reference)
"""ChebyKAN layer (degree-7) on 8 Trainium2 NeuronCores.

out[b,o] = sum_{i,d} T_d(tanh(x[b,i])) * C[o,i,d]  +  x @ BW.T

V2 strategy:
  - Data-parallel over batch: 16384 rows -> 8 cores x 2048.
  - T_0 == 1 contribution folded into a host-precomputed bias[o].
  - Cheby matmuls (7/8 of the FLOPs) run in fp8e4m3 with
    perf_mode=DoubleRow (2 fp8 MACs/cell/cycle, K=256 per matmul);
    coeffs are host-prescaled by 2**16 for fp8 representability.
    The base matmul runs in float32r with base_weight prescaled by
    the same 2**16 so both accumulate into one PSUM tile; the
    eviction rescales by 2**-16 and adds the bias.
  - Chebyshev basis is computed in bf16 on DVE (2x mode), cast to
    fp8 pair-interleaved tiles on ACT, once per batch super-tile
    (reused across both o-half passes).
  - out_features live on PSUM partitions: x ships pre-transposed
    (xT) and outT is transposed back on the host.
"""

import numpy as np

import concourse.mybir as mybir
from concourse import bacc, tile
from concourse.bass_utils import run_bass_kernel_spmd

IN_F = 1024
OUT_F = 1024
DEG = 7
N_CORES = 8
SC = float(2 ** 16)

F32 = mybir.dt.float32
F32R = mybir.dt.float32r
BF16 = mybir.dt.bfloat16
FP8 = mybir.dt.float8e4
ALU = mybir.AluOpType
ACTF = mybir.ActivationFunctionType
DR = mybir.MatmulPerfMode.DoubleRow


def _build_program(b_core: int, n_cores: int = N_CORES):
    bsup = min(1024, b_core)
    assert b_core % bsup == 0
    n_bs = b_core // bsup
    F = bsup
    n_half = (F + 511) // 512
    n_ci = IN_F // 128            # 8
    n_pair = n_ci // 2            # 4
    n_oh = 2

    nc = bacc.Bacc("TRN2", target_bir_lowering=False, debug=False,
                   num_devices=n_cores)
    xT = nc.dram_tensor("xT", [IN_F, b_core], F32R, kind="ExternalInput")
    w8 = nc.dram_tensor("w8", [n_oh, n_pair, 128, DEG * 2 * 512], FP8,
                        kind="ExternalInput")
    wb = nc.dram_tensor("wb", [n_oh, n_ci, 128, 512], F32R,
                        kind="ExternalInput")
    biasm = nc.dram_tensor("biasm", [128, 8], F32, kind="ExternalInput")
    outT = nc.dram_tensor("outT", [OUT_F, b_core], F32, kind="ExternalOutput")

    with tile.TileContext(nc) as tc:
        with (
            tc.tile_pool(name="const", bufs=1) as cpool,
            tc.tile_pool(name="xp", bufs=6) as xpool,
            tc.tile_pool(name="bwork", bufs=8) as kpool,
            tc.tile_pool(name="t8", bufs=7 * n_pair + 7) as t8pool,
            tc.tile_pool(name="w8p", bufs=3) as wpool,
            tc.tile_pool(name="wbp", bufs=10) as wbpool,
            tc.tile_pool(name="op", bufs=3) as opool,
            tc.tile_pool(name="ps", bufs=4, space="PSUM") as ppool,
        ):
            bias_sb = cpool.tile([128, 8], F32)
            nc.sync.dma_start(bias_sb[:], biasm[:, :])

            for bs in range(n_bs):
                # ---- Phase A: bf16 basis -> fp8, pair-fused [128, 2F] ----
                # Sign-flipped ADD-only recurrence (TT-SUBTRACT has no 2x
                # uop): V_d = (s_d/s_{d-1}) * W1 * V_{d-1} + V_{d-2} with
                # W1 = 2*tanh(x), N1 = -W1 (via tanh(-x)), V_d = s_d*2*T_d,
                # s = [+,-,-,+,+,-,-] for d=1..7. Host weights absorb
                # s_d/2. Each op covers both i-chunks of a DR pair.
                t8 = {}
                for pair in range(n_pair):
                    for d in range(1, DEG + 1):
                        t8[(pair, d)] = t8pool.tile(
                            [128, 2, F], FP8, tag="t8",
                            name=f"t8_{bs}_{pair}_{d}")
                xts = []
                wb0 = {}
                for pair in range(n_pair):
                    # per-plane x DMAs interleaved with oh=0 base-weight
                    # loads: the first base matmul needs just one x plane
                    # and one wb tile, so don't queue megabytes ahead of it
                    xt = xpool.tile([128, 2 * F], F32R, tag="x",
                                    name=f"x_{bs}_{pair}")
                    for plane in range(2):
                        ci = 2 * pair + plane
                        nc.sync.dma_start(
                            xt[:, plane * F:(plane + 1) * F],
                            xT[ci * 128:(ci + 1) * 128,
                               bs * F:(bs + 1) * F])
                        wbt = wbpool.tile([128, 512], F32R, tag="wb",
                                          name=f"wb0_{bs}_{ci}")
                        nc.sync.dma_start(wbt[:], wb[0, ci, :, :])
                        wb0[ci] = wbt
                    xts.append(xt)

                    def run_basis(pair, xt, cols):
                        """Recurrence + fp8 casts over a column slice of
                        both planes (cols within [0, F))."""
                        n = cols.stop - cols.start
                        W = 2 * n

                        def v3(t):  # [128, 2n] tile -> [128, 2, n] view
                            return t[:].rearrange("p (two f) -> p two f",
                                                  two=2)

                        xnb = kpool.tile([128, W], BF16, tag="bw")
                        xnn = kpool.tile([128, W], BF16, tag="bw")
                        for plane in range(2):
                            xsl = xt[:, plane * F + cols.start:
                                     plane * F + cols.stop].bitcast(F32)
                            osl = slice(plane * n, (plane + 1) * n)
                            nc.scalar.activation(xnb[:, osl], xsl,
                                                 ACTF.Tanh)
                            nc.scalar.activation(xnn[:, osl], xsl,
                                                 ACTF.Tanh, scale=-1.0)

                        def cast8(d, src):
                            nc.scalar.copy(t8[(pair, d)][:, :, cols],
                                           v3(src))

                        w1 = kpool.tile([128, W], BF16, tag="bw")
                        nc.vector.tensor_add(w1[:], xnb[:], xnb[:])
                        n1 = kpool.tile([128, W], BF16, tag="bw")
                        nc.vector.tensor_add(n1[:], xnn[:], xnn[:])
                        cast8(1, w1)
                        m2 = kpool.tile([128, W], BF16, tag="bw")
                        nc.vector.tensor_mul(m2[:], n1[:], w1[:])
                        v2 = kpool.tile([128, W], BF16, tag="bw")
                        nc.vector.tensor_scalar_add(v2[:], m2[:], 2.0)
                        cast8(2, v2)
                        prev2, prev1 = w1, v2
                        bmul = {3: w1, 4: n1, 5: w1, 6: n1, 7: w1}
                        for d in range(3, DEG + 1):
                            md = kpool.tile([128, W], BF16, tag="bw")
                            nc.vector.tensor_mul(md[:], bmul[d][:],
                                                 prev1[:])
                            vd = kpool.tile([128, W], BF16, tag="bw")
                            nc.vector.tensor_add(vd[:], md[:], prev2[:])
                            cast8(d, vd)
                            prev2, prev1 = prev1, vd

                    run_basis(pair, xt, slice(0, F))

                # ---- Phase B: matmuls ----
                for oh in range(n_oh):
                    po = [ppool.tile([128, F], F32, tag="ps",
                                     name=f"po_{bs}_{oh}_{i}")
                          for i in range(4)]
                    # Interleave base (fp32r, x-only) matmul sections
                    # between cheby pairs: the x-only work covers the
                    # basis production lag of the later pairs. For the
                    # very first pass there is no production lead at all,
                    # so spend the entire base section as runway first.
                    if bs == 0 and oh == 0:
                        base_before = {0: [0, 1, 2, 3, 4, 5], 1: [6],
                                       2: [7], 3: []}
                    else:
                        base_before = {p: [2 * p, 2 * p + 1]
                                       for p in range(n_pair)}
                    for pair in range(n_pair):
                        sect = base_before[pair]
                        wbts = {}
                        for ci in sect:
                            if oh == 0:
                                wbts[ci] = wb0[ci]
                            else:
                                wbt = wbpool.tile([128, 512], F32R,
                                                  tag="wb")
                                nc.sync.dma_start(wbt[:],
                                                  wb[oh, ci, :, :])
                                wbts[ci] = wbt
                        # o4-major order: po[3]'s first write of each pass
                        # comes ~3/4 of a section later, giving the prior
                        # pass's last eviction slack to free the slot
                        for o4 in range(4):
                            for ci in sect:
                                plane = ci % 2
                                for h in range(n_half):
                                    c0 = h * 512
                                    c1 = min(c0 + 512, F)
                                    nc.tensor.matmul(
                                        po[o4][:, c0:c1],
                                        wbts[ci][:, o4 * 128:
                                                 (o4 + 1) * 128],
                                        xts[ci // 2][:, plane * F + c0:
                                                     plane * F + c1],
                                        start=(ci == sect[0]
                                               and pair == 0),
                                        stop=False)
                        wm = wpool.tile([128, DEG * 2 * 512], FP8, tag="w8")
                        nc.sync.dma_start(wm[:], w8[oh, pair, :, :])
                        wmv = wm[:].rearrange("p (d two o) -> p d two o",
                                              d=DEG, two=2)
                        for o4 in range(4):
                            for d in range(1, DEG + 1):
                                lhsT = wmv[:, d - 1, :,
                                           o4 * 128:(o4 + 1) * 128]
                                for h in range(n_half):
                                    c0 = h * 512
                                    c1 = min(c0 + 512, F)
                                    nc.tensor.matmul(
                                        po[o4][:, c0:c1],
                                        lhsT,
                                        t8[(pair, d)][:, :, c0:c1],
                                        start=False,
                                        stop=(pair == n_pair - 1
                                              and d == DEG),
                                        perf_mode=DR)

                    for o4 in range(4):
                        oc = oh * 4 + o4
                        ob = opool.tile([128, F], F32, tag="o")
                        bias_col = bias_sb[:, oc:oc + 1]
                        # alternate eviction engines so the pass-boundary
                        # drain of 4 psum tiles is 2-wide, not serial on
                        # the DVE that also produces the next basis. The
                        # last tile is the end-gated one: drain it as two
                        # half-width ops on both engines in parallel.
                        if o4 == 3 and n_half == 2:
                            nc.scalar.activation(
                                ob[:, 0:512], po[o4][:, 0:512],
                                ACTF.Identity, bias=bias_col,
                                scale=1.0 / SC)
                            nc.vector.tensor_scalar(
                                ob[:, 512:F], po[o4][:, 512:F], 1.0 / SC,
                                bias_col, ALU.mult, ALU.add)
                            for h in range(2):
                                c0, c1 = h * 512, min((h + 1) * 512, F)
                                nc.sync.dma_start(
                                    outT[oc * 128:(oc + 1) * 128,
                                         bs * F + c0:bs * F + c1],
                                    ob[:, c0:c1])
                            continue
                        if o4 % 2 == 0:
                            nc.scalar.activation(
                                ob[:], po[o4][:], ACTF.Identity,
                                bias=bias_col, scale=1.0 / SC)
                        else:
                            nc.vector.tensor_scalar(
                                ob[:], po[o4][:], 1.0 / SC,
                                bias_col, ALU.mult, ALU.add)
                        nc.sync.dma_start(
                            outT[oc * 128:(oc + 1) * 128,
                                 bs * F:(bs + 1) * F], ob[:])
    nc.compile()
    return nc


def _prep_weights(cheby_coeffs: np.ndarray, base_weight: np.ndarray):
    C = np.asarray(cheby_coeffs, dtype=np.float32)
    BW = np.asarray(base_weight, dtype=np.float32)
    # cheby fp8 mega-tiles: [oh, pair, k, d(1..7), plane, o(512)].
    # The device basis tiles hold V_d = s_d * 2 * T_d, so fold s_d / 2
    # into the weights (s_d^2 == 1).
    sgn = np.array([0, 1, -1, -1, 1, 1, -1, -1], dtype=np.float32)
    Cs = (C * (sgn / 2.0 * SC)).reshape(2, 512, 4, 2, 128, DEG + 1)
    w8 = np.ascontiguousarray(
        Cs[:, :, :, :, :, 1:].transpose(0, 2, 4, 5, 3, 1)
    ).astype(mybir.dt.np(FP8))
    w8 = np.ascontiguousarray(w8.reshape(2, 4, 128, DEG * 2 * 512))
    # base fp32r: [oh, ci, k, o(512)], prescaled
    wbs = (BW.T * SC).reshape(8, 128, 2, 512)          # [ci,k,oh,o]
    wb = np.ascontiguousarray(wbs.transpose(2, 0, 1, 3))
    bias = C[:, :, 0].sum(axis=1)
    biasm = np.ascontiguousarray(bias.reshape(8, 128).T)
    return w8, wb, biasm


_PROGRAM_CACHE = {}


def _make_in_maps(x, cheby_coeffs, base_weight):
    x = np.asarray(x, dtype=np.float32)
    b_core = x.shape[0] // N_CORES
    w8, wb, biasm = _prep_weights(cheby_coeffs, base_weight)
    in_maps = []
    for c in range(N_CORES):
        xs = x[c * b_core:(c + 1) * b_core]
        in_maps.append({
            "xT": np.ascontiguousarray(xs.T),
            "w8": w8,
            "wb": wb,
            "biasm": biasm,
        })
    return in_maps


def kernel(x: np.ndarray, cheby_coeffs: np.ndarray,
           base_weight: np.ndarray) -> np.ndarray:
    x = np.asarray(x, dtype=np.float32)
    b_full = x.shape[0]
    assert b_full % N_CORES == 0
    b_core = b_full // N_CORES

    key = (b_core, N_CORES)
    if key not in _PROGRAM_CACHE:
        _PROGRAM_CACHE[key] = _build_program(b_core)
    nc = _PROGRAM_CACHE[key]

    in_maps = _make_in_maps(x, cheby_coeffs, base_weight)
    res = run_bass_kernel_spmd(nc, in_maps, core_ids=list(range(N_CORES)))
    out = np.empty((b_full, OUT_F), dtype=np.float32)
    for c in range(N_CORES):
        out[c * b_core:(c + 1) * b_core] = res.results[c]["outT"].T
    return out

